# revision 23
# baseline (speedup 1.0000x reference)
"""Center-contrast triplet loss on 8 Trainium2 NeuronCores — collective-free.

Feature-dim sharding: core m gets the m-th 256-wide feature slice of both
inputs with batch columns reordered k-major so every per-class K-sum is a
short chain of packed halving adds on the DVE. Inputs ship as fp8-e4m3
(half the HBM stream; final-scalar quantization error ~2e-4, well inside
the 2e-2 gate) EXCEPT the last x1 class-block span, which ships fp16: its
tree is the critical tail, and the DVE 2x fast path needs 2-byte
SAME-TILE packed operands (fp8 operands, cross-tile adds and strided
reduces all run 1x, measured).

Streaming schedule (two HWDGE queues, round-robin DMA engines):
  - x2 tiles ship as k-half chunks (one per queue) that land together;
    per-half stage-1 adds into one shared buffer, then same-tile
    halvings yield s2_t [128, 512].
  - x1 ships as class-block spans that shrink toward the end (q0q1 fp8,
    q2 fp8, q3 fp16), (t0, t1) pair per span landing together; 3-level
    DVE trees.
  - Per class block q: two accumulating PE matmuls (contraction =
    feature partitions, f32 PSUM) form Gram row-block q; ACT casts it to
    fp16 and ships it on the scalar queue. The LAST block is column-
    split into two PSUM tiles; DVE (idle by then) does both casts and
    the by-then-empty sync queue ships both halves.
  - ss = sum_p s2^2 ships as TWO single-shot ones-matmul rows (one per
    feature tile, summed on the host) so no accumulation chain ever
    waits at the tail; ab leaves mid-stream via the GpSimd queue.
  - pp = sum_p s1*s2 is NOT computed on device — it is exactly diag(G),
    read off the shipped Gram on the host.

No on-device collective (ncfw rendezvous ~75us >> 0.5 MB of data): every
core ships its partial Gram + ss rows; the host unshard sums the 8
partials and runs the trivial relu/rowmax/cummax/sum epilogue (values are
64x the true ones since centers are kept as sums-of-8; folded at the end).
"""

import numpy as np

import concourse.bacc as bacc
import concourse.mybir as mybir
import concourse.tile as tile
from concourse.bass_utils import run_bass_kernel_spmd
from concourse.vector_clock import ScopedClock


class LeanTileContext(tile.TileContext):
    """TileContext with a drain-only exit.

    The stock exit emits drain + all-engine EVSEM barrier + semaphore
    clears + second barrier. The runtime re-arms semaphores at NEFF
    load/execute, so for this single-shot kernel a drain (which already
    waits on every engine's clock) is sufficient; verified correct across
    repeated executions of the same NEFF.
    """

    def _drain_and_barrier(self, tick_clock, wait_clock):
        drain_inst = self.nc.sync.drain()
        wait_clock.add_sem_waits(
            drain_inst.ins, ScopedClock({None: tick_clock.global_clock})
        )
        popped = self.nc._tile_sem_poison_stack.pop()
        assert popped is self._sem_poison
        sems = list(self.sems.allocated().values())
        sem_nums = [s.num if hasattr(s, "num") else s for s in sems]
        self.nc._state.prepend_free_semaphores(sem_nums)
        for poison_set in self.nc._tile_sem_poison_stack:
            poison_set.update(sem_nums)


N_CORES = 8
B, D, C, K = 4096, 2048, 512, 8
DS = D // N_CORES          # 256 features per core -> 2 partition tiles
NQ = 4                     # class blocks of 128
QC = C // NQ               # 128 classes per block
F32 = mybir.dt.float32
F16 = mybir.dt.float16
BF16 = mybir.dt.bfloat16

# x1 chunking: class-block spans, big early, small fp16 tail
X1_SPANS = [(0, 2), (2, 3), (3, 4)]
F8 = mybir.dt.float8e4


def build_nc():
    nc = bacc.Bacc(
        "TRN2", target_bir_lowering=False, debug=False, num_devices=N_CORES
    )
    # x2 columns k-major over all classes (k*C + c); tile t0 ships fp8
    # (first to arrive -> earliest DVE start), t1 fp16 (2x stage-1 adds)
    x2a = nc.dram_tensor("x2a", [DS // 2, B], F8, kind="ExternalInput")
    x2b = nc.dram_tensor("x2b", [DS // 2, B], F16, kind="ExternalInput")
    # x1 columns: per span, k-major within span; fp16 for 2x adds
    x1t = nc.dram_tensor("x1t", [DS, B], F16, kind="ExternalInput")
    v = nc.dram_tensor("v", [C, C], F16, kind="ExternalOutput")
    ab = nc.dram_tensor("ab", [1, 2 * C], F32, kind="ExternalOutput")

    with LeanTileContext(nc) as tc:
        with (
            tc.tile_pool(name="sbuf", bufs=1) as pool,
            tc.tile_pool(name="psum", bufs=1, space="PSUM") as psum,
        ):
            const_f32 = pool.tile([128, 1], F32, name="const_f32")
            nc.vector.memset(const_f32[:], 1.0)
            ones_col = pool.tile([128, 1], BF16, name="ones_col")
            nc.vector.tensor_copy(ones_col[:], const_f32[:])

            # tiny first DMAs warm both HWDGE queues before the big stream
            warm_a = pool.tile([1, 64], F8, name="warm_a")
            nc.sync.dma_start(warm_a[:], x2a[0:1, 0:64])
            warm_b = pool.tile([1, 64], F16, name="warm_b")
            nc.scalar.dma_start(warm_b[:], x1t[0:1, 0:64])

            # x2 t0: four fp8 k-quarter chunks (k-pair each, 1KB desc)
            x2_t0q = []
            for qi in range(4):
                eng = nc.sync if qi % 2 == 0 else nc.scalar
                xt = pool.tile([128, B // 4], F8, name=f"x2_0q{qi}")
                eng.dma_start(
                    xt[:], x2a[:, (B // 4) * qi : (B // 4) * (qi + 1)]
                )
                x2_t0q.append(xt)
            # x2 t1: two fp16 k-half chunks (4KB desc)
            x2_t1h = []
            for h, eng in ((0, nc.sync), (1, nc.scalar)):
                xt = pool.tile([128, B // 2], F16, name=f"x2_1h{h}")
                eng.dma_start(
                    xt[:], x2b[:, (B // 2) * h : (B // 2) * (h + 1)]
                )
                x2_t1h.append(xt)

            # x1 span chunks, (t0, span) on sync / (t1, span) on scalar
            x1_ts = {}
            for si, (q0, q1) in enumerate(X1_SPANS):
                w = K * QC * (q1 - q0)
                for t, eng in ((0, nc.sync), (1, nc.scalar)):
                    xq = pool.tile([128, w], F16, name=f"x1_{t}s{si}")
                    eng.dma_start(
                        xq[:],
                        x1t[
                            128 * t : 128 * (t + 1),
                            K * QC * q0 : K * QC * q1,
                        ],
                    )
                    x1_ts[t, si] = xq

            g_ps = [
                psum.tile([128, C], F32, name=f"g{q}", tag="gps", bufs=NQ - 1)
                for q in range(NQ - 1)
            ]
            # last block column-split over two PSUM tiles for a short tail
            g3 = [
                psum.tile([128, C // 2], F32, name=f"g3{i}", tag="g3", bufs=2)
                for i in range(2)
            ]
            ss_ab = [
                psum.tile([1, C], F32, name=f"ss_{t}", tag="ss", bufs=2)
                for t in range(2)
            ]

            def tree3(src, w, tag):
                """3-level packed halving-add K-sum: [128, w] -> [128, w//8]."""
                r1 = pool.tile([128, w // 2], F16, name=f"r1_{tag}")
                nc.vector.tensor_tensor(
                    r1[:], src[:, : w // 2], src[:, w // 2 :],
                    op=mybir.AluOpType.add,
                )
                r2 = pool.tile([128, w // 4], F16, name=f"r2_{tag}")
                nc.vector.tensor_tensor(
                    r2[:], r1[:, : w // 4], r1[:, w // 4 :],
                    op=mybir.AluOpType.add,
                )
                s = pool.tile([128, w // 8], BF16, name=f"s_{tag}")
                nc.vector.tensor_tensor(
                    s[:], r2[:, : w // 8], r2[:, w // 8 :],
                    op=mybir.AluOpType.add,
                )
                return s

            with nc.allow_low_precision(reason="16-bit centers, f32 accum"):
                # s2 trees: stage-1 adds into ONE buffer per tile,
                # then same-tile halvings -> s2_t [128, 512]
                s2_t, sq_t = [], []
                # t0: four fp8 quarters, st1 = k-pair add per quarter
                r1_0 = pool.tile([128, B // 2], F16, name="x2r1_0")
                for qi in range(4):
                    src0 = x2_t0q[qi]
                    nc.vector.tensor_tensor(
                        r1_0[:, (B // 8) * qi : (B // 8) * (qi + 1)],
                        src0[:, : B // 8], src0[:, B // 8 :],
                        op=mybir.AluOpType.add,
                    )
                r2_0 = pool.tile([128, B // 4], F16, name="x2r2_0")
                nc.vector.tensor_tensor(
                    r2_0[:], r1_0[:, : B // 4], r1_0[:, B // 4 :],
                    op=mybir.AluOpType.add,
                )
                s2_0 = pool.tile([128, C], BF16, name="s2_0")
                nc.vector.tensor_tensor(
                    s2_0[:], r2_0[:, :C], r2_0[:, C:], op=mybir.AluOpType.add
                )
                s2_t.append(s2_0)
                # t1: two fp16 halves at the 2x rate
                r1_1 = pool.tile([128, B // 2], F16, name="x2r1_1")
                for h in range(2):
                    src1 = x2_t1h[h]
                    nc.vector.tensor_tensor(
                        r1_1[:, (B // 4) * h : (B // 4) * (h + 1)],
                        src1[:, : B // 4], src1[:, B // 4 :],
                        op=mybir.AluOpType.add,
                    )
                r2_1 = pool.tile([128, B // 4], F16, name="x2r2_1")
                nc.vector.tensor_tensor(
                    r2_1[:], r1_1[:, : B // 4], r1_1[:, B // 4 :],
                    op=mybir.AluOpType.add,
                )
                s2_1 = pool.tile([128, C], BF16, name="s2_1")
                nc.vector.tensor_tensor(
                    s2_1[:], r2_1[:, :C], r2_1[:, C:], op=mybir.AluOpType.add
                )
                s2_t.append(s2_1)
                for t in range(2):
                    sq = pool.tile([128, C], BF16, name=f"sq_{t}")
                    nc.scalar.square(sq[:], s2_t[t][:])
                    sq_t.append(sq)
                    # single-shot ss row for this tile; host sums the two
                    nc.tensor.matmul(
                        ss_ab[t][:], lhsT=ones_col[:], rhs=sq[:],
                        start=True, stop=True,
                    )

                # ab assembly + DMA leave mid-stream on ACT/GpSimd
                ab_sb = pool.tile([1, 2 * C], F32, name="ab_sb")
                for t in range(2):
                    nc.scalar.copy(ab_sb[:, C * t : C * (t + 1)], ss_ab[t][:])
                nc.gpsimd.dma_start(ab[:], ab_sb[:])

                last_q = X1_SPANS[-1][1] - 1
                for si, (q0, q1) in enumerate(X1_SPANS):
                    w = K * QC * (q1 - q0)
                    s1_t = [
                        tree3(x1_ts[t, si], w, f"x1_{t}s{si}") for t in range(2)
                    ]
                    for q in range(q0, q1):
                        bs = slice(QC * (q - q0), QC * (q - q0 + 1))
                        if q == last_q:
                            # column-split tail: 2 PSUM tiles, DVE casts,
                            # both halves shipped on the idle sync queue
                            for t in range(2):
                                for i in range(2):
                                    nc.tensor.matmul(
                                        g3[i][:],
                                        lhsT=s1_t[t][:, bs],
                                        rhs=s2_t[t][:, C // 2 * i : C // 2 * (i + 1)],
                                        start=(t == 0), stop=(t == 1),
                                    )
                            for i in range(2):
                                v_sb = pool.tile(
                                    [128, C // 2], F16, name=f"v_sb3{i}"
                                )
                                nc.vector.tensor_copy(v_sb[:], g3[i][:])
                                nc.sync.dma_start(
                                    v[
                                        QC * q : QC * (q + 1),
                                        C // 2 * i : C // 2 * (i + 1),
                                    ],
                                    v_sb[:],
                                )
                        else:
                            for t in range(2):
                                nc.tensor.matmul(
                                    g_ps[q][:],
                                    lhsT=s1_t[t][:, bs],
                                    rhs=s2_t[t][:],
                                    start=(t == 0), stop=(t == 1),
                                )
                            v_sb = pool.tile([128, C], F16, name=f"v_sb{q}")
                            nc.scalar.copy(v_sb[:], g_ps[q][:])
                            nc.scalar.dma_start(
                                v[QC * q : QC * (q + 1), :], v_sb[:]
                            )

    nc.finalize()
    return nc


def prepare_in_maps(input1, input2):
    import ml_dtypes

    f8 = ml_dtypes.float8_e4m3
    x1 = np.asarray(input1, dtype=np.float32)
    x2 = np.asarray(input2, dtype=np.float32)
    # x2: [D, B] with cols k-major over all classes: col = k*C + c;
    # even 128-row tiles ship fp8, odd ones fp16
    x2t = np.ascontiguousarray(
        x2.T.reshape(D, C, K).transpose(0, 2, 1).reshape(D, B)
    )
    x2r = x2t.reshape(D // 128, 128, B)
    x2a = x2r[0::2].reshape(-1, B).astype(f8)
    x2b = x2r[1::2].reshape(-1, B).astype(np.float16)
    # x1: [D, B] span-major, k-major within each span, fp16
    xr = x1.T.reshape(D, NQ, QC, K)
    cols = []
    for q0, q1 in X1_SPANS:
        slab = xr[:, q0:q1]                      # [D, nq, QC, K]
        cols.append(slab.transpose(0, 3, 1, 2).reshape(D, -1))
    x1t = np.ascontiguousarray(
        np.concatenate(cols, axis=1), dtype=np.float16
    )
    in_maps = []
    for m in range(N_CORES):
        in_maps.append(
            {
                "x2a": x2a[128 * m : 128 * (m + 1)],
                "x2b": x2b[128 * m : 128 * (m + 1)],
                "x1t": x1t[m * DS : (m + 1) * DS],
            }
        )
    return in_maps


def postprocess(results):
    g = np.zeros((C, C), dtype=np.float32)
    ss = np.zeros(C, dtype=np.float64)
    for m in range(N_CORES):
        g += np.asarray(results[m]["v"], dtype=np.float32)
        a = np.asarray(results[m]["ab"], dtype=np.float64).reshape(2 * C)
        ss += a[:C] + a[C:]
    pp = np.diag(g).astype(np.float64)           # pp_i = G_ii = s1_i . s2_i
    a_col = 0.5 * ss - pp          # per-row bias
    b_row = 0.5 * ss               # per-col bias
    vfull = g + (a_col[:, None] - b_row[None, :]).astype(np.float32)
    rm = np.maximum(vfull.max(axis=1), 0.0) / 32.0
    return np.float32(np.maximum.accumulate(rm).sum())


_NC_CACHE = None


def kernel(input1, input2, targets1, targets2):
    global _NC_CACHE
    if _NC_CACHE is None:
        _NC_CACHE = build_nc()
    in_maps = prepare_in_maps(input1, input2)
    res = run_bass_kernel_spmd(_NC_CACHE, in_maps, list(range(N_CORES)))
    return postprocess(res.results)


# revision 24
# speedup vs baseline: 1.0110x; 1.0110x over previous
"""Center-contrast triplet loss on 8 Trainium2 NeuronCores — collective-free.

Feature-dim sharding: core m gets the m-th 256-wide feature slice of both
inputs with batch columns reordered k-major so every per-class K-sum is a
short chain of packed halving adds on the DVE. Inputs ship as fp8-e4m3
(half the HBM stream; final-scalar quantization error ~2e-4, well inside
the 2e-2 gate) EXCEPT the last x1 class-block span, which ships fp16: its
tree is the critical tail, and the DVE 2x fast path needs 2-byte
SAME-TILE packed operands (fp8 operands, cross-tile adds and strided
reduces all run 1x, measured).

Streaming schedule (two HWDGE queues, round-robin DMA engines):
  - x2 tiles ship as k-half chunks (one per queue) that land together;
    per-half stage-1 adds into one shared buffer, then same-tile
    halvings yield s2_t [128, 512].
  - x1 ships as class-block spans that shrink toward the end (q0q1 fp8,
    q2 fp8, q3 fp16), (t0, t1) pair per span landing together; 3-level
    DVE trees.
  - Per class block q: two accumulating PE matmuls (contraction =
    feature partitions, f32 PSUM) form Gram row-block q; ACT casts it to
    fp16 and ships it on the scalar queue. The LAST block is column-
    split into two PSUM tiles; DVE (idle by then) does both casts and
    the by-then-empty sync queue ships both halves.
  - ss = sum_p s2^2 ships as TWO single-shot ones-matmul rows (one per
    feature tile, summed on the host) so no accumulation chain ever
    waits at the tail; ab leaves mid-stream via the GpSimd queue.
  - pp = sum_p s1*s2 is NOT computed on device — it is exactly diag(G),
    read off the shipped Gram on the host.

No on-device collective (ncfw rendezvous ~75us >> 0.5 MB of data): every
core ships its partial Gram + ss rows; the host unshard sums the 8
partials and runs the trivial relu/rowmax/cummax/sum epilogue (values are
64x the true ones since centers are kept as sums-of-8; folded at the end).
"""

import numpy as np

import concourse.bacc as bacc
import concourse.mybir as mybir
import concourse.tile as tile
from concourse.bass_utils import run_bass_kernel_spmd
from concourse.vector_clock import ScopedClock


class LeanTileContext(tile.TileContext):
    """TileContext with a drain-only exit.

    The stock exit emits drain + all-engine EVSEM barrier + semaphore
    clears + second barrier. The runtime re-arms semaphores at NEFF
    load/execute, so for this single-shot kernel a drain (which already
    waits on every engine's clock) is sufficient; verified correct across
    repeated executions of the same NEFF.
    """

    def _drain_and_barrier(self, tick_clock, wait_clock):
        drain_inst = self.nc.sync.drain()
        wait_clock.add_sem_waits(
            drain_inst.ins, ScopedClock({None: tick_clock.global_clock})
        )
        popped = self.nc._tile_sem_poison_stack.pop()
        assert popped is self._sem_poison
        sems = list(self.sems.allocated().values())
        sem_nums = [s.num if hasattr(s, "num") else s for s in sems]
        self.nc._state.prepend_free_semaphores(sem_nums)
        for poison_set in self.nc._tile_sem_poison_stack:
            poison_set.update(sem_nums)


N_CORES = 8
B, D, C, K = 4096, 2048, 512, 8
DS = D // N_CORES          # 256 features per core -> 2 partition tiles
NQ = 4                     # class blocks of 128
QC = C // NQ               # 128 classes per block
F32 = mybir.dt.float32
F16 = mybir.dt.float16
BF16 = mybir.dt.bfloat16

# x1 chunking: class-block spans, big early, small fp16 tail
X1_SPANS = [(0, 2), (2, 3), (3, 4)]
F8 = mybir.dt.float8e4


def build_nc():
    nc = bacc.Bacc(
        "TRN2", target_bir_lowering=False, debug=False, num_devices=N_CORES
    )
    # x2t columns: k-major over all classes (k*C + c)
    x2t = nc.dram_tensor("x2t", [DS, B], F8, kind="ExternalInput")
    # x1t columns: per span, k-major within span (k*(nq*QC) + c_span)
    x1t = nc.dram_tensor("x1t", [DS, 3 * K * QC], F8, kind="ExternalInput")
    # fp16 tail span: its tree is on the critical tail, 2x path needed
    x1tail = nc.dram_tensor("x1tail", [DS, K * QC], F16, kind="ExternalInput")
    v = nc.dram_tensor("v", [C, C], F16, kind="ExternalOutput")
    ab = nc.dram_tensor("ab", [1, 2 * C], F32, kind="ExternalOutput")

    with LeanTileContext(nc) as tc:
        with (
            tc.tile_pool(name="sbuf", bufs=1) as pool,
            tc.tile_pool(name="psum", bufs=1, space="PSUM") as psum,
        ):
            const_f32 = pool.tile([128, 1], F32, name="const_f32")
            nc.vector.memset(const_f32[:], 1.0)
            ones_col = pool.tile([128, 1], BF16, name="ones_col")
            nc.vector.tensor_copy(ones_col[:], const_f32[:])

            # tiny first DMAs warm both HWDGE queues before the big stream
            warm_a = pool.tile([1, 64], F8, name="warm_a")
            nc.sync.dma_start(warm_a[:], x2t[0:1, 0:64])
            warm_b = pool.tile([1, 64], F8, name="warm_b")
            nc.scalar.dma_start(warm_b[:], x1t[0:1, 0:64])

            # x2 k-half chunks: h0 = k 0..3 on sync, h1 = k 4..7 on scalar
            x2_th = {}
            for t in range(2):
                for h, eng in ((0, nc.sync), (1, nc.scalar)):
                    xt = pool.tile([128, B // 2], F8, name=f"x2_{t}{h}")
                    eng.dma_start(
                        xt[:],
                        x2t[128 * t : 128 * (t + 1), (B // 2) * h : (B // 2) * (h + 1)],
                    )
                    x2_th[t, h] = xt

            # x1 span chunks, (t0, span) on sync / (t1, span) on scalar;
            # spans 0-1 fp8 from x1t, tail span fp16 from x1tail
            x1_ts = {}
            for si, (q0, q1) in enumerate(X1_SPANS):
                w = K * QC * (q1 - q0)
                tail = q1 == NQ
                for t, eng in ((0, nc.sync), (1, nc.scalar)):
                    xq = pool.tile(
                        [128, w], F16 if tail else F8, name=f"x1_{t}s{si}"
                    )
                    if tail:
                        eng.dma_start(
                            xq[:], x1tail[128 * t : 128 * (t + 1), :]
                        )
                    else:
                        eng.dma_start(
                            xq[:],
                            x1t[
                                128 * t : 128 * (t + 1),
                                K * QC * q0 : K * QC * q1,
                            ],
                        )
                    x1_ts[t, si] = xq

            g_ps = [
                psum.tile([128, C], F32, name=f"g{q}", tag="gps", bufs=NQ - 1)
                for q in range(NQ - 1)
            ]
            # last block column-split over two PSUM tiles for a short tail
            g3 = [
                psum.tile([128, C // 2], F32, name=f"g3{i}", tag="g3", bufs=2)
                for i in range(2)
            ]
            ss_ab = [
                psum.tile([1, C], F32, name=f"ss_{t}", tag="ss", bufs=2)
                for t in range(2)
            ]

            def tree3(src, w, tag):
                """3-level packed halving-add K-sum: [128, w] -> [128, w//8]."""
                r1 = pool.tile([128, w // 2], F16, name=f"r1_{tag}")
                nc.vector.tensor_tensor(
                    r1[:], src[:, : w // 2], src[:, w // 2 :],
                    op=mybir.AluOpType.add,
                )
                r2 = pool.tile([128, w // 4], F16, name=f"r2_{tag}")
                nc.vector.tensor_tensor(
                    r2[:], r1[:, : w // 4], r1[:, w // 4 :],
                    op=mybir.AluOpType.add,
                )
                s = pool.tile([128, w // 8], BF16, name=f"s_{tag}")
                nc.vector.tensor_tensor(
                    s[:], r2[:, : w // 8], r2[:, w // 8 :],
                    op=mybir.AluOpType.add,
                )
                return s

            with nc.allow_low_precision(reason="16-bit centers, f32 accum"):
                # s2 trees: per-half stage-1 into ONE buffer, same-tile halvings
                s2_t, sq_t = [], []
                for t in range(2):
                    r1 = pool.tile([128, B // 2], F16, name=f"x2r1_{t}")
                    for h in range(2):
                        src = x2_th[t, h]
                        nc.vector.tensor_tensor(
                            r1[:, (B // 4) * h : (B // 4) * (h + 1)],
                            src[:, : B // 4], src[:, B // 4 :],
                            op=mybir.AluOpType.add,
                        )
                    r2 = pool.tile([128, B // 4], F16, name=f"x2r2_{t}")
                    nc.vector.tensor_tensor(
                        r2[:], r1[:, : B // 4], r1[:, B // 4 :],
                        op=mybir.AluOpType.add,
                    )
                    s2 = pool.tile([128, C], BF16, name=f"s2_{t}")
                    nc.vector.tensor_tensor(
                        s2[:], r2[:, :C], r2[:, C:], op=mybir.AluOpType.add
                    )
                    s2_t.append(s2)
                    sq = pool.tile([128, C], BF16, name=f"sq_{t}")
                    nc.scalar.square(sq[:], s2[:])
                    sq_t.append(sq)
                    # single-shot ss row for this tile; host sums the two
                    nc.tensor.matmul(
                        ss_ab[t][:], lhsT=ones_col[:], rhs=sq[:],
                        start=True, stop=True,
                    )

                # ab assembly + DMA leave mid-stream on ACT/GpSimd
                ab_sb = pool.tile([1, 2 * C], F32, name="ab_sb")
                for t in range(2):
                    nc.scalar.copy(ab_sb[:, C * t : C * (t + 1)], ss_ab[t][:])
                nc.gpsimd.dma_start(ab[:], ab_sb[:])

                last_q = X1_SPANS[-1][1] - 1
                for si, (q0, q1) in enumerate(X1_SPANS):
                    w = K * QC * (q1 - q0)
                    s1_t = [
                        tree3(x1_ts[t, si], w, f"x1_{t}s{si}") for t in range(2)
                    ]
                    for q in range(q0, q1):
                        bs = slice(QC * (q - q0), QC * (q - q0 + 1))
                        if q == last_q:
                            # column-split tail: 2 PSUM tiles, DVE casts,
                            # both halves shipped on the idle sync queue
                            for t in range(2):
                                for i in range(2):
                                    nc.tensor.matmul(
                                        g3[i][:],
                                        lhsT=s1_t[t][:, bs],
                                        rhs=s2_t[t][:, C // 2 * i : C // 2 * (i + 1)],
                                        start=(t == 0), stop=(t == 1),
                                    )
                            for i in range(2):
                                v_sb = pool.tile(
                                    [128, C // 2], F16, name=f"v_sb3{i}"
                                )
                                nc.vector.tensor_copy(v_sb[:], g3[i][:])
                                nc.sync.dma_start(
                                    v[
                                        QC * q : QC * (q + 1),
                                        C // 2 * i : C // 2 * (i + 1),
                                    ],
                                    v_sb[:],
                                )
                        else:
                            for t in range(2):
                                nc.tensor.matmul(
                                    g_ps[q][:],
                                    lhsT=s1_t[t][:, bs],
                                    rhs=s2_t[t][:],
                                    start=(t == 0), stop=(t == 1),
                                )
                            v_sb = pool.tile([128, C], F16, name=f"v_sb{q}")
                            nc.scalar.copy(v_sb[:], g_ps[q][:])
                            nc.scalar.dma_start(
                                v[QC * q : QC * (q + 1), :], v_sb[:]
                            )

    nc.finalize()
    return nc


def prepare_in_maps(input1, input2):
    import ml_dtypes

    f8 = ml_dtypes.float8_e4m3
    x1 = np.asarray(input1, dtype=np.float32)
    x2 = np.asarray(input2, dtype=np.float32)
    # x2: [D, B] with cols k-major over all classes: col = k*C + c
    x2t = np.ascontiguousarray(
        x2.T.reshape(D, C, K).transpose(0, 2, 1).reshape(D, B)
    ).astype(f8)
    # x1: span-major, k-major within each span; fp8 spans 0-1, fp16 tail
    xr = x1.T.reshape(D, NQ, QC, K)
    cols = []
    for q0, q1 in X1_SPANS[:-1]:
        slab = xr[:, q0:q1]                      # [D, nq, QC, K]
        cols.append(slab.transpose(0, 3, 1, 2).reshape(D, -1))
    x1t = np.concatenate(cols, axis=1).astype(f8)
    q0, q1 = X1_SPANS[-1]
    x1tail = np.ascontiguousarray(
        xr[:, q0:q1].transpose(0, 3, 1, 2).reshape(D, -1), dtype=np.float16
    )
    in_maps = []
    for m in range(N_CORES):
        sl = slice(m * DS, (m + 1) * DS)
        in_maps.append(
            {"x1t": x1t[sl], "x2t": x2t[sl], "x1tail": x1tail[sl]}
        )
    return in_maps


def postprocess(results):
    g = np.zeros((C, C), dtype=np.float32)
    ss = np.zeros(C, dtype=np.float64)
    for m in range(N_CORES):
        g += np.asarray(results[m]["v"], dtype=np.float32)
        a = np.asarray(results[m]["ab"], dtype=np.float64).reshape(2 * C)
        ss += a[:C] + a[C:]
    pp = np.diag(g).astype(np.float64)           # pp_i = G_ii = s1_i . s2_i
    a_col = 0.5 * ss - pp          # per-row bias
    b_row = 0.5 * ss               # per-col bias
    vfull = g + (a_col[:, None] - b_row[None, :]).astype(np.float32)
    rm = np.maximum(vfull.max(axis=1), 0.0) / 32.0
    return np.float32(np.maximum.accumulate(rm).sum())


_NC_CACHE = None


def kernel(input1, input2, targets1, targets2):
    global _NC_CACHE
    if _NC_CACHE is None:
        _NC_CACHE = build_nc()
    in_maps = prepare_in_maps(input1, input2)
    res = run_bass_kernel_spmd(_NC_CACHE, in_maps, list(range(N_CORES)))
    return postprocess(res.results)
